# revision 27
# baseline (speedup 1.0000x reference)
"""AttentionPooling GNN kernel for 8 Trainium2 NeuronCores.

Strategy (v3)
-------------
Graph-parallel sharding: 128 graphs -> 16 per core.  Host does index
preprocessing only (edge permutation, slot packing, weight folding).

Device algorithm (per core):
 1. Edge stream in fp8e4 (e4m3), packed into 256-edge chunks (2 interleaved
    128-partition planes).  A shared triangular stationary tri2 (fp8) with
    DoubleRow perf mode computes 2-edge-slot prefix sums P4 of whole chunks
    in one matmul per 2048-edge supergroup: out [128 slots, 8 chunks x 64
    feats] in PSUM at 0.5 cycles/row.
 2. P4 is cast to a bf16 SBUF table [128 part = slot, rank = chunk, 128]
    (payload 64 feats + 64 garbage pad to make 256B gather elements).
 3. Per node: A[n] = P4[hi_n] - P4[lo_n] where lo_n == hi_{n-1} (nodes are
    packed in order, pads contribute 0), so ONE chained gather of hi rows
    per 8-node-chunk batch serves both phi (cols 1..nn) and plo (cols
    0..nn-1).  Chunk-crossing nodes (prefix resets per chunk) get their plo
    zeroed via a host-shipped 0/1 mask.  SBUF-source transpose dma_gather
    -> feature-major [64, nodes] directly (no PE transposes).
 4. meanA_T = (phi - plo*mask) * inv_deg (vector, feature-major).  Dense
    per node chunk k: psum = hT[:,k] @ W1 + meanA_T[:,k] @ WA  (2 matmuls,
    260 wide: [v(256) | scores(4)]).  Constant rows (biases, has-edge when
    all deg>0) fold into the output bias (score part cancels in softmax).
 5. w = exp(scores); pr = [w*v | w] bf16; pool matmul with one-hot graph
    membership accumulates [16, 260] segment sums in PSUM (pool matmul is
    software-pipelined one chunk behind the vs matmuls).
 6. pooled = U/denom; out = pooled @ out_w.T + ob_eff (fp32).
"""
import sys

sys.path.insert(0, "/opt/trn_rl_repo")

import numpy as np

NUM_HEADS = 4
G_TOTAL = 128
CORES = 8
GL = G_TOTAL // CORES       # graphs per core
P = 128                     # partitions
SLOT = 2                    # edges per slot
SPC = 128                   # slots per chunk (256 edges)
CPS = 8                     # chunks per supergroup (2048 edges)
SG_E = CPS * SPC * SLOT     # 2048 edges per supergroup
KB_PER_BATCH = 8            # node chunks per gather batch
DENSE_LAG = 9               # supergroups of lag before dense emission


def _batch_splits(nkb):
    """Batch sizes: 8s up front, smaller at the tail for earlier overlap."""
    out = []
    left = nkb
    while left > 10:
        out.append(8)
        left -= 8
    if left > 6:
        out.append(4)
        left -= 4
    while left > 2:
        out.append(min(4, left - 2))
        left -= out[-1]
    if left:
        out.append(left)
    return out


# ----------------------------------------------------------------- host prep
def _pack_core(deg):
    """Assign each node a run of ceil(deg/2) slots, in node order; runs never
    straddle a 128-slot chunk.  Returns (s0, r, end_cursor)."""
    NL = len(deg)
    r = (deg + SLOT - 1) // SLOT
    s0 = np.zeros(NL, np.int64)
    cur = 0
    for n in range(NL):
        rn = r[n]
        if rn == 0:
            s0[n] = -1
            continue
        in_c = cur % SPC
        if in_c + rn > SPC:
            cur = (cur // SPC + 1) * SPC
        s0[n] = cur
        cur += rn
    return s0, r, cur


def _prep(edge_index, batch):
    """Shard + pack.  Returns per-core dict of host index arrays + config."""
    row = np.asarray(edge_index[0], np.int64)
    batch = np.asarray(batch, np.int64)
    gstart = np.searchsorted(batch, np.arange(G_TOTAL + 1))
    order = np.argsort(row, kind="stable")
    row_s = row[order]

    cores = []
    max_sg = 0
    max_nl = 0
    for c in range(CORES):
        n0, n1 = int(gstart[GL * c]), int(gstart[GL * (c + 1)])
        NL = n1 - n0
        max_nl = max(max_nl, NL)
        e0, e1 = np.searchsorted(row_s, [n0, n1])
        eord = order[e0:e1]
        lrow = row_s[e0:e1] - n0
        deg = np.bincount(lrow, minlength=NL)
        s0, r, cur = _pack_core(deg)
        n_sg = (cur + SG_E // SLOT - 1) // (SG_E // SLOT)
        max_sg = max(max_sg, n_sg)
        cores.append(dict(n0=n0, n1=n1, NL=NL, eord=eord, lrow=lrow,
                          deg=deg, s0=s0, r=r))
    NSG = max_sg
    NKB = (max_nl + P - 1) // P
    NC_NODES = NKB * P
    splits = _batch_splits(NKB)
    NB = len(splits)

    batches = []
    k0 = 0
    for sz in splits:
        k1 = k0 + sz
        nn = sz * P
        batches.append(dict(k0=k0, k1=k1, nn=nn, nidx=nn + P))
        k0 = k1

    for c, st in enumerate(cores):
        deg, s0, r = st["deg"], st["s0"], st["r"]
        NL = st["NL"]
        # edge stream positions
        first_edge = np.concatenate([[0], np.cumsum(deg)])[:-1]
        has = deg > 0
        epos_s0 = np.repeat(s0[has], deg[has])
        within = np.arange(len(st["lrow"])) - np.repeat(first_edge[has], deg[has])
        slot = epos_s0 + within // SLOT
        posin = within % SLOT
        e_in_chunk = (slot % SPC) * SLOT + posin
        chunk = slot // SPC
        sg = chunk // CPS
        cc = chunk % CPS
        i = e_in_chunk // P
        p = e_in_chunk % P
        erow = sg * 2048 + p * 16 + i * 8 + cc
        assert erow.max(initial=0) < NSG * 2048
        st["erow"] = erow

        # hi tokens per node (token id == global slot id).  deg-0 and pad
        # nodes chain to the previous node's hi (A = 0, in-window).
        ends = np.where(s0 >= 0, s0 + r - 1, -1)
        prev_hi = np.maximum(np.maximum.accumulate(
            np.concatenate([[0], ends[:-1]])), 0)
        hi_tok = np.where(s0 >= 0, s0 + r - 1, prev_hi)
        # plo validity: plo (= prev node's hi value) is the true in-chunk
        # prefix before this node's run only when both are in the same chunk
        # and the run doesn't start the chunk; otherwise prefix-before = 0.
        mask = np.where(
            s0 >= 0,
            ((s0 % SPC != 0) & (prev_hi // SPC == s0 // SPC)).astype(np.float32),
            1.0)
        last = int(np.maximum.accumulate(np.where(ends >= 0, ends, 0))[-1]) \
            if NL > 0 else 0
        hi_tok = np.pad(hi_tok, (0, NC_NODES - NL), constant_values=last)
        mask = np.pad(mask, (0, NC_NODES - NL), constant_values=1.0)
        st["hi_tok"] = hi_tok
        st["mask"] = mask

    for b, bt in enumerate(batches):
        base = 10 ** 9
        end = 0
        for st in cores:
            lead = st["hi_tok"][max(bt["k0"] * P - 1, 0)]
            toks = st["hi_tok"][bt["k0"] * P:bt["k1"] * P]
            base = min(base, int(lead) // P)
            end = max(end, int(toks.max()) // P)
        bt["base_rank"] = base
        bt["win"] = end - base + 1
        bt["sg_ready"] = end // CPS
        assert bt["win"] * P < 32767

    cfg = dict(NSG=NSG, NKB=NKB, NC_NODES=NC_NODES, NB=NB, batches=batches)
    return cores, cfg


def _wrap_idx(a):
    """[M] -> [128, M//16] int16, F-wrapped 16-row block replicated 8x."""
    m = a.reshape(-1, 16).T.astype(np.int16)
    return np.tile(m, (8, 1))


def _fold_weights(node_w, node_b, edge_w, edge_b, query, in_w, in_b,
                  out_w, out_b, all_deg_pos):
    D = query.shape[-1]
    dh = D // NUM_HEADS
    wq, wk, wv = in_w[:D], in_w[D:2 * D], in_w[2 * D:]
    bq, bk, bv = in_b[:D], in_b[D:2 * D], in_b[2 * D:]
    q = (query[0] @ wq.T + bq).reshape(NUM_HEADS, dh)
    s_w = np.einsum("hj,hjd->dh", q, wk.reshape(NUM_HEADS, dh, D)) / np.sqrt(dh)
    M2 = np.concatenate([wv.T, s_w], axis=1)          # [256, 260]
    W1 = (node_w.T @ M2)                               # [128, 260]
    WA = (edge_w.T @ M2)                               # [64, 260]
    # constant term: node bias row + value bias; has-edge row if all deg>0
    w_one_v = node_b @ wv.T + bv                       # [256]
    w_he = edge_b @ M2                                 # [260]
    assert all_deg_pos or not np.any(edge_b), \
        "isolated nodes with nonzero edge bias not supported"
    if all_deg_pos:
        w_one_v = w_one_v + w_he[:256]
    ob_eff = out_b + w_one_v @ out_w.T                 # [256]
    return (W1.astype(np.float32), WA.astype(np.float32),
            ob_eff.astype(np.float32))


# ------------------------------------------------------- numpy device model
def _numpy_device_model(cores, cfg, streams, in_maps):
    import ml_dtypes
    bf = lambda x: x.astype(ml_dtypes.bfloat16).astype(np.float32)
    NSG, NKB, NC_NODES = cfg["NSG"], cfg["NKB"], cfg["NC_NODES"]
    outs = []
    for c, st in enumerate(cores):
        im = in_maps[c]
        stream = streams[c].astype(np.float32)
        P4 = np.zeros((NSG * CPS * SPC, 64), np.float32)
        for sgi in range(NSG):
            by_pc = stream[sgi * 2048:(sgi + 1) * 2048].reshape(P, 2, CPS, 64)
            for cc in range(CPS):
                chunk = np.concatenate([by_pc[:, 0, cc], by_pc[:, 1, cc]])
                pre = np.add.reduceat(chunk, np.arange(0, 256, SLOT), 0).cumsum(0)
                t0 = (sgi * CPS + cc) * SPC
                P4[t0:t0 + SPC] = pre
        tab = bf(P4)
        hi_tok = st["hi_tok"]
        phi = tab[hi_tok]
        lo_tok = np.concatenate([[hi_tok[0]], hi_tok[:-1]])
        plo = tab[lo_tok] * st["mask"][:, None]
        inv = im["_inv"][:, None]
        meanA_T = bf((phi - plo) * inv)
        hT = im["nd1"][:, :, :P].astype(np.float32)
        W1 = im["w1"].astype(np.float32)
        WA = im["wa"].astype(np.float32)
        memall = im["nd1"][:, :, P:].astype(np.float32)
        U = np.zeros((GL, 260), np.float32)
        for k in range(NKB):
            hk = hT[:, k, :].T
            mk = meanA_T[k * P:(k + 1) * P]
            vs = hk @ W1 + mk @ WA
            w = np.exp(vs[:, 256:])
            prv = bf(w[:, :, None] * vs[:, :256].reshape(-1, 4, 64)).reshape(-1, 256)
            prw = bf(w)
            U += memall[:, k, :].T @ np.concatenate([prv, prw], 1)
        den = np.maximum(U[:, 256:], 1e-30)
        pooled = U[:, :256].reshape(GL, 4, 64) / den[:, :, None]
        o = pooled.reshape(GL, 256) @ im["_owt_f"] + im["ob"]
        outs.append(o)
    return np.concatenate(outs).reshape(G_TOTAL, 1, 256)


# ------------------------------------------------------------- bass program
def _build_program(cfg):
    import concourse.bacc as bacc
    import concourse.mybir as mybir
    import concourse.tile as tile

    F32 = mybir.dt.float32
    BF16 = mybir.dt.bfloat16
    FP8 = mybir.dt.float8e4
    I16 = mybir.dt.int16
    AF = mybir.ActivationFunctionType
    NSG, NKB, NB = cfg["NSG"], cfg["NKB"], cfg["NB"]
    batches = cfg["batches"]
    NRANKS = NSG * CPS
    TOTIDX = sum(bt["nidx"] for bt in batches)

    nc = bacc.Bacc("TRN2", num_devices=CORES, num_swdge_queues=4)
    es_d = nc.dram_tensor("es", [NSG, P, 16, 64], FP8, kind="ExternalInput")
    tri_d = nc.dram_tensor("tri", [P, 2, SPC], FP8, kind="ExternalInput")
    nd1_d = nc.dram_tensor("nd1", [P, NKB, P + GL], BF16, kind="ExternalInput")
    nd2_d = nc.dram_tensor("nd2", [64, NKB, 2, P], BF16, kind="ExternalInput")
    idx_d = nc.dram_tensor("idx", [P, TOTIDX // 16], I16, kind="ExternalInput")
    w1_d = nc.dram_tensor("w1", [P, 260], BF16, kind="ExternalInput")
    wa_d = nc.dram_tensor("wa", [64, 260], BF16, kind="ExternalInput")
    idtf_d = nc.dram_tensor("idtf", [GL, GL], F32, kind="ExternalInput")
    owt_d = nc.dram_tensor("owt", [256, 256], F32, kind="ExternalInput")
    ob_d = nc.dram_tensor("ob", [GL, 256], F32, kind="ExternalInput")
    y_d = nc.dram_tensor("y", [GL, 256], F32, kind="ExternalOutput")

    with tile.TileContext(nc) as tc:
        with tc.tile_pool(name="const", bufs=1) as cp, \
             tc.tile_pool(name="sb", bufs=3) as sb, \
             tc.tile_pool(name="ps", bufs=2, space="PSUM") as ps, \
             tc.tile_pool(name="pacc", bufs=1, space="PSUM") as pacc:

            trib = cp.tile([P, 2, SPC], FP8, name="trib")
            nc.sync.dma_start(out=trib[:], in_=tri_d.ap()[:, :, :])
            w1 = cp.tile([P, 260], BF16, name="w1")
            wa = cp.tile([64, 260], BF16, name="wa")
            idtf = cp.tile([GL, GL], F32, name="idtf")
            owt = cp.tile([P, 2, 256], F32, name="owt")
            obt = cp.tile([GL, 256], F32, name="obt")
            idxt = cp.tile([P, TOTIDX // 16], I16, name="idxt")
            nc.scalar.dma_start(out=idxt[:], in_=idx_d.ap()[:, :])
            nc.scalar.dma_start(out=w1[:], in_=w1_d.ap()[:, :])
            nc.scalar.dma_start(out=wa[:], in_=wa_d.ap()[:, :])

            # big per-node tensors are loaded just-in-time per batch (on the
            # scalar HWDGE queue) to keep DMA bandwidth free for the edge
            # stream early on; packed so each batch is 2 dma_starts
            nd1 = cp.tile([P, NKB, P + GL], BF16, name="nd1")
            nd2 = cp.tile([64, NKB, 2, P], BF16, name="nd2")

            def emit_node_loads(b):
                bt = batches[b]
                k0, k1 = bt["k0"], bt["k1"]
                nc.scalar.dma_start(out=nd1[:, k0:k1, :],
                                    in_=nd1_d.ap()[:, k0:k1, :])
                nc.scalar.dma_start(out=nd2[:, k0:k1, :, :],
                                    in_=nd2_d.ap()[:, k0:k1, :, :])

            table = cp.tile([P, NRANKS, 2, 64], BF16, name="table")
            gouts = [None] * NB

            pool_ps = pacc.tile([GL, 260], F32, name="pool_ps")

            idx_off = [0]
            for bt in batches:
                idx_off.append(idx_off[-1] + bt["nidx"])

            def emit_gather(b):
                bt = batches[b]
                g = sb.tile([P, bt["nidx"]], BF16, name=f"gout{b}",
                            tag="gout", bufs=3)
                gouts[b] = g
                nc.gpsimd.dma_gather(
                    out_ap=g[:].rearrange("p (one n) -> p one n", one=1),
                    in_ap=table[:, bt["base_rank"]:bt["base_rank"] + bt["win"], :, :],
                    idxs_ap=idxt[:, idx_off[b] // 16:idx_off[b + 1] // 16],
                    num_idxs=bt["nidx"], num_idxs_reg=bt["nn"] + 1,
                    elem_size=P,
                    transpose=True, single_packet=False, queue_num=b % 4,
                    sbuf_tokens_per_rank=128,
                    sbuf_free_dim_per_rank=256)

            pending_pool = []   # chunks whose pool matmul is deferred

            def flush_pool(keep=0):
                while len(pending_pool) > keep:
                    pk, ppr = pending_pool.pop(0)
                    nc.tensor.matmul(out=pool_ps[:], lhsT=nd1[:, pk, P:],
                                     rhs=ppr[:], start=(pk == 0),
                                     stop=(pk == NKB - 1))

            def emit_dense(b):
                bt = batches[b]
                k0, k1 = bt["k0"], bt["k1"]
                nn = bt["nn"]
                g = gouts[b]
                kb = k1 - k0
                # meanA_T = (phi - plo*mask) * inv, feature-major bf16
                t = sb.tile([64, kb, P], BF16, name="t", tag="t", bufs=2)
                nc.vector.tensor_tensor(
                    out=t[:],
                    in0=g[:64, 0:nn].rearrange("q (k n) -> q k n", n=P),
                    in1=nd2[:, k0:k1, 1, :],
                    op=mybir.AluOpType.mult)
                d = sb.tile([64, kb, P], BF16, name="d", tag="d", bufs=2)
                nc.vector.tensor_sub(
                    out=d[:],
                    in0=g[:64, 1:nn + 1].rearrange("q (k n) -> q k n", n=P),
                    in1=t[:])
                ma = sb.tile([64, kb, P], BF16, name="ma", tag="ma", bufs=2)
                nc.vector.tensor_tensor(
                    out=ma[:], in0=d[:],
                    in1=nd2[:, k0:k1, 0, :],
                    op=mybir.AluOpType.mult)
                # vs PSUM is a 2-bank pair tile: two chunks share one exp ACT
                for k0p in range(k0, k1, 2):
                    kk = [k0p] if k0p + 1 >= k1 else [k0p, k0p + 1]
                    vsp = ps.tile([P, 2, 512], F32, name="vsp", tag="vs", bufs=2)
                    for ii, k in enumerate(kk):
                        j = k - k0
                        nc.tensor.matmul(out=vsp[:, ii, :260],
                                         lhsT=nd1[:, k, :P], rhs=w1[:],
                                         start=True, stop=False)
                        nc.tensor.matmul(out=vsp[:, ii, :260],
                                         lhsT=ma[:, j, :],
                                         rhs=wa[:], start=False, stop=True)
                    prp = sb.tile([P, 2, 260], BF16, name="prp", tag="pr",
                                  bufs=3)
                    nc.scalar.activation(
                        out=prp[:, :len(kk), 256:260],
                        in_=vsp[:, :len(kk), 256:260], func=AF.Exp)
                    for ii, k in enumerate(kk):
                        nc.vector.tensor_tensor(
                            out=prp[:, ii, :256].rearrange(
                                "p (h f) -> p h f", h=NUM_HEADS),
                            in0=vsp[:, ii, :256].rearrange(
                                "p (h f) -> p h f", h=NUM_HEADS),
                            in1=prp[:, ii, 256:260].broadcast_to(
                                [P, NUM_HEADS, 64]),
                            op=mybir.AluOpType.mult)
                        flush_pool(keep=2)
                        pending_pool.append((k, prp[:, ii, :]))

            next_g = 0
            next_d = 0
            for sg0 in range(0, NSG, 2):
                npair = min(2, NSG - sg0)
                et2 = sb.tile([P, 2, 16, 64], FP8, name="et2", tag="et", bufs=3)
                nc.sync.dma_start(
                    out=et2[:, :npair, :, :],
                    in_=es_d.ap()[sg0:sg0 + npair, :, :, :]
                        .rearrange("s p i f -> p s i f"))
                for sg in range(sg0, sg0 + npair):
                    pp = ps.tile([P, 512], F32, name="pp", tag="pp", bufs=3)
                    nc.tensor.matmul(
                        out=pp[:],
                        lhsT=trib[:],
                        rhs=et2[:, sg - sg0, :, :]
                            .rearrange("p (i c) f -> p i (c f)", i=2),
                        start=True, stop=True,
                        perf_mode=mybir.MatmulPerfMode.DoubleRow)
                    if sg < 10 or sg % 4 == 3:
                        nc.vector.tensor_scalar_mul(
                            out=table[:, sg * CPS:(sg + 1) * CPS, 0, :],
                            in0=pp[:].rearrange("p (c f) -> p c f", c=CPS),
                            scalar1=1.0)
                    else:
                        nc.scalar.copy(
                            out=table[:, sg * CPS:(sg + 1) * CPS, 0, :],
                            in_=pp[:].rearrange("p (c f) -> p c f", c=CPS))
                    while next_g < NB and batches[next_g]["sg_ready"] <= sg:
                        emit_node_loads(next_g)
                        emit_gather(next_g)
                        next_g += 1
                    while (next_d < next_g and
                           batches[next_d]["sg_ready"] + DENSE_LAG <= sg):
                        emit_dense(next_d)
                        next_d += 1
            while next_g < NB:
                emit_node_loads(next_g)
                emit_gather(next_g)
                next_g += 1
            while next_d < NB:
                emit_dense(next_d)
                next_d += 1
            flush_pool()

            nc.scalar.dma_start(out=idtf[:], in_=idtf_d.ap()[:, :])
            nc.scalar.dma_start(
                out=owt[:],
                in_=owt_d.ap()[:, :].rearrange("(i p) f -> p i f", p=P))
            nc.scalar.dma_start(out=obt[:], in_=ob_d.ap()[:, :])

            # ---- final: normalize + output projection
            den = sb.tile([GL, 4], F32, name="den")
            nc.vector.tensor_scalar_max(out=den[:], in0=pool_ps[:, 256:260],
                                        scalar1=1e-30)
            rden = sb.tile([GL, 4], F32, name="rden")
            nc.vector.reciprocal(out=rden[:], in_=den[:])
            pn = sb.tile([GL, 256], F32, name="pn")
            for hh in range(NUM_HEADS):
                nc.vector.tensor_scalar_mul(out=pn[:, 64 * hh:64 * hh + 64],
                                            in0=pool_ps[:, 64 * hh:64 * hh + 64],
                                            scalar1=rden[:, hh:hh + 1])
            pnT = sb.tile([P, 2, GL], F32, name="pnT")
            for i in range(2):
                ptp = ps.tile([P, GL], F32, name="ptp", tag="pp", bufs=3)
                nc.tensor.transpose(out=ptp[:], in_=pn[:, i * P:(i + 1) * P],
                                    identity=idtf[:])
                nc.vector.tensor_copy(out=pnT[:, i, :], in_=ptp[:])
            ops_t = ps.tile([GL, 256], F32, name="ops_t", tag="pp", bufs=3)
            for i in range(2):
                nc.tensor.matmul(out=ops_t[:], lhsT=pnT[:, i, :], rhs=owt[:, i, :],
                                 start=(i == 0), stop=(i == 1))
            osb = sb.tile([GL, 256], F32, name="osb")
            nc.vector.tensor_add(out=osb[:], in0=ops_t[:], in1=obt[:])
            nc.sync.dma_start(out=y_d.ap()[:, :], in_=osb[:])

    nc.finalize()
    return nc


_CACHE = {}


def _get_program(cfg):
    key = (cfg["NSG"], cfg["NKB"],
           tuple((bt["base_rank"], bt["win"], bt["sg_ready"], bt["nidx"])
                 for bt in cfg["batches"]))
    if key not in _CACHE:
        _CACHE[key] = _build_program(cfg)
    return _CACHE[key]


def kernel(h, edge_index, edge_attr, batch, num_graphs,
           node_w, node_b, edge_w, edge_b, query, in_w, in_b, out_w, out_b,
           _trace=False, _numpy_only=False):
    import ml_dtypes
    bf16 = ml_dtypes.bfloat16
    fp8 = ml_dtypes.float8_e4m3
    h = np.asarray(h, np.float32)
    edge_attr = np.asarray(edge_attr, np.float32)
    batch_np = np.asarray(batch, np.int64)
    assert int(num_graphs) == G_TOTAL

    cores, cfg = _prep(edge_index, batch_np)
    NSG, NKB, NC_NODES = cfg["NSG"], cfg["NKB"], cfg["NC_NODES"]
    all_deg_pos = all((st["deg"] > 0).all() for st in cores)
    W1, WA, ob_eff = _fold_weights(
        np.asarray(node_w, np.float32), np.asarray(node_b, np.float32),
        np.asarray(edge_w, np.float32), np.asarray(edge_b, np.float32),
        np.asarray(query, np.float32), np.asarray(in_w, np.float32),
        np.asarray(in_b, np.float32), np.asarray(out_w, np.float32),
        np.asarray(out_b, np.float32), all_deg_pos)

    tri = ((np.arange(2)[None, :, None] * P + np.arange(P)[:, None, None])
           <= (SLOT * np.arange(SPC)[None, None, :] + SLOT - 1))
    shared = dict(
        tri=tri.astype(fp8),
        w1=W1.astype(bf16), wa=WA.astype(bf16),
        idtf=np.eye(GL, dtype=np.float32),
        owt=np.ascontiguousarray(np.asarray(out_w, np.float32).T),
        ob=np.tile(ob_eff[None, :], (GL, 1)),
    )
    batches = cfg["batches"]
    in_maps = []
    streams = []
    for c, st in enumerate(cores):
        NL = st["NL"]
        stream = np.zeros((NSG * 2048, 64), np.float32)
        stream[st["erow"]] = edge_attr[st["eord"]].astype(fp8).astype(np.float32)
        streams.append(stream.astype(fp8))
        es = streams[-1].reshape(NSG, P, 16, 64)

        hpad = np.zeros((NC_NODES, P), np.float32)
        hpad[:NL] = h[st["n0"]:st["n1"]]
        ht3 = hpad.reshape(NKB, P, P).transpose(2, 0, 1)

        deg = np.pad(st["deg"], (0, NC_NODES - NL)).astype(np.float32)
        inv = (1.0 / np.maximum(deg, 1.0)).astype(np.float32)

        bl = np.full(NC_NODES, -1, np.int64)
        bl[:NL] = batch_np[st["n0"]:st["n1"]] - GL * c
        on = (bl[:, None] == np.arange(GL)[None, :]).astype(np.float32)
        mem3 = on.reshape(NKB, P, GL).transpose(1, 0, 2)

        nd1 = np.concatenate([ht3, mem3], axis=2).astype(bf16)
        nd2 = np.ascontiguousarray(np.broadcast_to(
            np.stack([inv.reshape(NKB, P), st["mask"].reshape(NKB, P)],
                     axis=1)[None, :, :, :], (64, NKB, 2, P))).astype(bf16)

        idx_parts = []
        hi_tok = st["hi_tok"]
        for bt in batches:
            k0, k1 = bt["k0"], bt["k1"]
            lead = hi_tok[max(k0 * P - 1, 0)]
            gidx = np.concatenate([[lead], hi_tok[k0 * P:k1 * P]])
            rel = gidx - bt["base_rank"] * P
            assert rel.min() >= 0 and rel.max() < bt["win"] * P, (c, k0)
            rel = np.pad(rel, (0, bt["nidx"] - len(rel)),
                         constant_values=-1)
            idx_parts.append(_wrap_idx(rel))
        idx = np.concatenate(idx_parts, axis=1)

        in_maps.append(dict(es=es, nd1=nd1, nd2=nd2, idx=idx, **shared))

    if _numpy_only:
        model_maps = [dict(im, _owt_f=np.asarray(out_w, np.float32).T,
                           _inv=(1.0 / np.maximum(
                               np.pad(st["deg"], (0, NC_NODES - st["NL"])),
                               1.0)).astype(np.float32))
                      for im, st in zip(in_maps, cores)]
        return _numpy_device_model(cores, cfg, streams, model_maps)

    from concourse.bass_utils import run_bass_kernel_spmd
    nc = _get_program(cfg)
    res = run_bass_kernel_spmd(nc, in_maps, core_ids=list(range(CORES)),
                               trace=_trace)
    out = np.concatenate([np.asarray(res.results[c]["y"], np.float32)
                          for c in range(CORES)])
    kernel._last_result = res
    return out.reshape(G_TOTAL, 1, 256)
